# revision 54
# baseline (speedup 1.0000x reference)
"""Causal multi-head attention (B=4, T=2048, D=2048, H=16) on 8 Trainium2
NeuronCores via Bass/Tile, SPMD with zero collectives.

Sharding: head-split tensor parallelism. Core pair (2b, 2b+1) handles batch
b; core 2b computes heads 0-7, core 2b+1 heads 8-15 (identical instruction
streams -- the head split is just a different weight slice). Each core:
  - projects Q^T/K^T (own heads, all T positions) and V (own heads) from a
    host-pre-transposed, bf16-cast x^T,
  - runs the full causal triangle for its 8 heads with suffix-window score
    tiles (keys tile j attends to the contiguous query suffix [128j, T)),
    only diagonal 128x128 blocks need a mask multiply,
  - emits a PARTIAL output projection o_part = A_own @ Wo[own rows, :].
The host sums the two partials of each pair and adds bo during unshard.

Numerics: all matmul operands bf16 (hosts pre-casts x/W; on-chip
intermediates are cast to bf16 during PSUM evacuation), f32 PSUM
accumulation throughout; exp without max-subtraction (scores are O(1));
bk dropped (softmax-invariant); bv folded into the post-softmax normalize.
Max rel err vs f32 reference ~3.5e-3 (numpy bit-exact simulation).
"""
import numpy as np
import ml_dtypes

import concourse.bacc as bacc
import concourse.mybir as mybir
from concourse.tile import TileContext
from concourse.bass_utils import run_bass_kernel_spmd

F32 = mybir.dt.float32
BF16 = mybir.dt.bfloat16
EXP = mybir.ActivationFunctionType.Exp
MULT = mybir.AluOpType.mult

PROD_CFG = dict(B=4, T=2048, D=2048, H=16)


def _derived(cfg):
    B, T, D, H = cfg["B"], cfg["T"], cfg["D"], cfg["H"]
    d = dict(cfg)
    d.update(
        DH=128,
        HH=H // 2,             # own heads per core
        DO=D // 2,             # own output-dim slice (HH * DH)
        DK=D // 128,           # contraction chunks over D
        QH=T // 2,             # query-half width for PSUM blocking
        N_CORES=2 * B,
    )
    return d


def _qq_units(qq):
    """Schedule for query quarter qq (512 cols): list of (jb, c0, diag).

    Key tile jb attends query cols [c0, 512) of the quarter; the 4 tiles
    at the causal diagonal get a mask multiply on their first 128 cols.
    """
    full = [(j, 0, False) for j in range(4 * qq)]
    diag = [(4 * qq + j, 128 * j, True) for j in range(4)]
    return full + diag


def build_nc(cfg):
    c = _derived(cfg)
    T, D = c["T"], c["D"]
    HH, DO, DK, QH = c["HH"], c["DO"], c["DK"], c["QH"]
    SCALE = float(c["DH"] ** -0.5)

    nc = bacc.Bacc(
        "TRN2", target_bir_lowering=False, debug=False, num_devices=c["N_CORES"]
    )
    xt = nc.dram_tensor("xt", [D, T], BF16, kind="ExternalInput").ap()
    wq = nc.dram_tensor("wq", [D, DO], BF16, kind="ExternalInput").ap()
    wk = nc.dram_tensor("wk", [D, DO], BF16, kind="ExternalInput").ap()
    wv = nc.dram_tensor("wv", [D, DO], BF16, kind="ExternalInput").ap()
    wo = nc.dram_tensor("wo", [DO, D], BF16, kind="ExternalInput").ap()
    bq = nc.dram_tensor("bq", [DO], F32, kind="ExternalInput").ap()
    bv = nc.dram_tensor("bv", [DO], F32, kind="ExternalInput").ap()
    mask_in = nc.dram_tensor("mask", [128, 128], BF16, kind="ExternalInput").ap()
    ones_in = nc.dram_tensor("ones_c", [128, 1], BF16, kind="ExternalInput").ap()
    o = nc.dram_tensor("o", [T, D], F32, kind="ExternalOutput").ap()

    with TileContext(nc) as tc:
        with (
            tc.tile_pool(name="const", bufs=1) as pconst,
            tc.tile_pool(name="kqv", bufs=1) as pkqv,
        ):
            mask_sb = pconst.tile([128, 128], BF16, tag="mask")
            nc.sync.dma_start(out=mask_sb[:], in_=mask_in[:])
            # touch Exp early so the ACT table load doesn't stall phase C
            warm = pconst.tile([1, 1], F32, tag="warm")
            nc.scalar.activation(warm[:], mask_sb[0:1, 0:1], EXP)
            ones_col = pconst.tile([128, 1], BF16, tag="ones_col")
            nc.sync.dma_start(out=ones_col[:], in_=ones_in[:])
            bq_sb = pconst.tile([128, HH], F32, tag="bq")
            nc.sync.dma_start(out=bq_sb[:], in_=bq.rearrange("(m p) -> p m", p=128))
            bv_sb = pconst.tile([128, HH], F32, tag="bv")
            nc.sync.dma_start(out=bv_sb[:], in_=bv.rearrange("(m p) -> p m", p=128))

            kt_sb = pkqv.tile([128, HH, T], BF16, tag="kt")
            qt_sb = pkqv.tile([128, HH, T], BF16, tag="qt")
            v_sb = pkqv.tile([128, T // 128, DO], BF16, tag="v")

            # ---------------- phase B: Q^T, K^T, V projections ----------------
            with (
                tc.tile_pool(name="pxt", bufs=1) as pxt,
                tc.tile_pool(name="pw", bufs=2) as pw,
                tc.tile_pool(name="ps_p", bufs=3, space="PSUM") as ps_p,
            ):
                # x^T on the scalar queue so weight DMAs (sync queue) are
                # not stuck behind the 8MB load; k-chunk matmuls start as
                # soon as their chunk lands.
                # one tile PER k-chunk so the first projection matmuls only
                # wait on their own chunk's DMA, not the whole 8MB load
                xt_r = xt.rearrange("(k p) t -> p k t", p=128)
                qs = [nc.scalar, nc.gpsimd]
                xtk = []
                for k in range(DK):
                    t_ = pxt.tile([128, T], BF16, tag=f"xt{k}")
                    xtk.append(t_)
                # chunk DMAs are issued from inside the FIRST projection
                # group's k-loop, right before the matmul that consumes
                # each chunk, so the emitted waits gate per-chunk and the
                # m=0 group overlaps the 8MB load instead of trailing it
                xt_started = False
                # Q^T / K^T: out[dh_p, t], stationary = weight chunk
                for w_in, is_q in ((wq, True), (wk, False)):
                    for m in range(HH):
                        wm = pw.tile([128, DK, 128], BF16, tag="wm")
                        nc.sync.dma_start(
                            out=wm[:],
                            in_=w_in.rearrange("(k p) n -> p k n", p=128)[
                                :, :, m * 128:(m + 1) * 128
                            ],
                        )
                        for tcn in range(T // 512):
                            ps = ps_p.tile([128, 512], F32, tag="psp")
                            for k in range(DK):
                                if not xt_started:
                                    qs[k % 2].dma_start(
                                        out=xtk[k][:], in_=xt_r[:, k, :]
                                    )
                                nc.tensor.matmul(
                                    ps[:],
                                    wm[:, k, :],
                                    xtk[k][:, tcn * 512:(tcn + 1) * 512],
                                    start=(k == 0),
                                    stop=(k == DK - 1),
                                )
                            xt_started = True
                            if is_q:
                                nc.vector.tensor_scalar_add(
                                    qt_sb[:, m, tcn * 512:(tcn + 1) * 512],
                                    ps[:], bq_sb[:, m:m + 1],
                                )
                            else:
                                nc.scalar.copy(
                                    kt_sb[:, m, tcn * 512:(tcn + 1) * 512], ps[:]
                                )
                # V: out[t_p, n], stationary = xt chunk, moving = wv
                for nb in range(DO // 512):
                    wvn = pw.tile([128, DK, 512], BF16, tag="wvn")
                    nc.sync.dma_start(
                        out=wvn[:],
                        in_=wv.rearrange("(k p) n -> p k n", p=128)[
                            :, :, nb * 512:(nb + 1) * 512
                        ],
                    )
                    for tt in range(T // 128):
                        ps = ps_p.tile([128, 512], F32, tag="psp")
                        for k in range(DK):
                            nc.tensor.matmul(
                                ps[:],
                                xtk[k][:, tt * 128:(tt + 1) * 128],
                                wvn[:, k, :],
                                start=(k == 0),
                                stop=(k == DK - 1),
                            )
                        nc.vector.tensor_copy(
                            v_sb[:, tt, nb * 512:(nb + 1) * 512], ps[:]
                        )

            # ---------------- phase C: attention per head ----------------
            with (
                tc.tile_pool(name="pat", bufs=1) as pat,
                tc.tile_pool(name="pwo", bufs=1) as pwo,
            ):
              at_sb = pat.tile([128, HH, T], BF16, tag="at")
              # prefetch Wo on an idle queue so phase D starts immediately
              wo_sb = pwo.tile([128, HH, D], BF16, tag="wo")
              for k in range(HH):
                  nc.gpsimd.dma_start(
                      out=wo_sb[:, k, :],
                      in_=wo.rearrange("(k p) n -> p k n", p=128)[:, k, :],
                  )
              with (
                  tc.tile_pool(name="ppt", bufs=7) as ppt,
                  tc.tile_pool(name="psm", bufs=3) as psm,
                  tc.tile_pool(name="ps_s", bufs=3, space="PSUM") as ps_s,
                  tc.tile_pool(name="ps_a", bufs=1, space="PSUM") as ps_a,
                  tc.tile_pool(name="ps_l", bufs=1, space="PSUM") as ps_l,
              ):
                QQ = 512

                def make_evac(psa, psl, h, q0):
                    def evac():
                        # evacuate + normalize (off PE critical path)
                        l_raw = psm.tile([1, QQ], F32, tag="lraw")
                        nc.vector.tensor_copy(l_raw[:], psl[:])
                        at_raw = psm.tile([128, QQ], F32, tag="atraw")
                        nc.vector.tensor_copy(at_raw[:], psa[:])
                        l_inv = psm.tile([1, QQ], F32, tag="linv")
                        nc.vector.reciprocal_approx_fast(l_inv[:], l_raw[:])
                        lb = psm.tile([128, QQ], F32, tag="lb")
                        nc.gpsimd.partition_broadcast(
                            lb[:], l_inv[:], channels=128
                        )
                        at_tmp = psm.tile([128, QQ], F32, tag="attmp")
                        nc.vector.tensor_tensor(
                            at_tmp[:], at_raw[:], lb[:], MULT
                        )
                        nc.vector.tensor_scalar_add(
                            at_sb[:, h, q0:q0 + QQ], at_tmp[:],
                            bv_sb[:, h:h + 1],
                        )
                    return evac

                # Global FIFO of deferred work (consume closures + evac
                # sentinels) flowing ACROSS quarter boundaries, so the PE
                # keeps issuing the next quarter's score matmuls while the
                # previous quarter's diag-tail exps drain on ACT.
                fifo = []

                def drain(k):
                    while len(fifo) > k:
                        fifo.pop(0)()

                for h in range(HH):
                    for qq in range(4):
                        q0 = qq * QQ
                        units = _qq_units(qq)
                        psa = ps_a.tile([128, QQ], F32, tag="psa")
                        psl = ps_l.tile([1, QQ], F32, tag="psl")
                        n_units = len(units)

                        def consume(pos, jb, c0, pt_ap,
                                    psa=psa, psl=psl, h=h, n=n_units):
                            vt = v_sb[:, jb, h * 128:(h + 1) * 128]
                            nc.tensor.matmul(
                                psa[:, c0:QQ],
                                vt,
                                pt_ap[:, c0:QQ],
                                start=(pos == 0),
                                stop=(pos == n - 1),
                            )
                            nc.tensor.matmul(
                                psl[:, c0:QQ],
                                ones_col[:],
                                pt_ap[:, c0:QQ],
                                start=(pos == 0),
                                stop=(pos == n - 1),
                            )

                        # two units share one [128, 2*QQ] pss tile and ONE
                        # wide exp — phase C is ACT-bound, and each ACT op
                        # carries ~370ns of fixed PSUM/SBUF access overhead
                        for pi in range(0, n_units, 2):
                            (jb0, c00, dg0), (jb1, c01, dg1) = \
                                units[pi], units[pi + 1]
                            pss = ps_s.tile([128, 2 * QQ], F32, tag="pss")
                            pt = ppt.tile([128, 2 * QQ], BF16, tag="pt")
                            for sub, (jb, c0) in enumerate(
                                ((jb0, c00), (jb1, c01))
                            ):
                                nc.tensor.matmul(
                                    pss[:, sub * QQ + c0:(sub + 1) * QQ],
                                    kt_sb[:, h, jb * 128:(jb + 1) * 128],
                                    qt_sb[:, h, q0 + c0:q0 + QQ],
                                    start=True, stop=True,
                                )
                            nc.scalar.activation(
                                pt[:, c00:2 * QQ], pss[:, c00:2 * QQ],
                                EXP, scale=SCALE,
                            )
                            if dg0:
                                nc.vector.tensor_mul(
                                    pt[:, c00:c00 + 128], pt[:, c00:c00 + 128],
                                    mask_sb[:],
                                )
                            if dg1:
                                nc.vector.tensor_mul(
                                    pt[:, QQ + c01:QQ + c01 + 128],
                                    pt[:, QQ + c01:QQ + c01 + 128],
                                    mask_sb[:],
                                )
                            fifo.append(lambda a=(pi, jb0, c00, pt[:, :QQ]),
                                        f=consume: f(*a))
                            fifo.append(lambda a=(pi + 1, jb1, c01, pt[:, QQ:]),
                                        f=consume: f(*a))
                            drain(9)
                        fifo.append(make_evac(psa, psl, h, q0))
                drain(0)

              # ---------------- phase D: partial output projection ----------------
              with (
                  tc.tile_pool(name="post", bufs=3) as post,
                  tc.tile_pool(name="ps_o", bufs=3, space="PSUM") as ps_o,
              ):
                  for tt in range(T // 128):
                      for cc in range(D // 512):
                          pso = ps_o.tile([128, 512], F32, tag="pso")
                          for k in range(HH):
                              nc.tensor.matmul(
                                  pso[:],
                                  at_sb[:, k, tt * 128:(tt + 1) * 128],
                                  wo_sb[:, k, cc * 512:(cc + 1) * 512],
                                  start=(k == 0),
                                  stop=(k == HH - 1),
                              )
                          ost = post.tile([128, 512], F32, tag="ost")
                          nc.scalar.copy(ost[:], pso[:])
                          nc.sync.dma_start(
                              out=o[tt * 128:(tt + 1) * 128,
                                    cc * 512:(cc + 1) * 512],
                              in_=ost[:],
                          )
    nc.compile()
    return nc


def host_shard(cfg, x_full, inputs):
    """Per-core input maps (head-split TP: core 2b+z = batch b, heads z*8..)."""
    c = _derived(cfg)
    B, DO = c["B"], c["DO"]
    bf = ml_dtypes.bfloat16
    f32 = np.float32
    mask = np.triu(np.ones((128, 128), dtype=f32)).astype(bf)
    ones_c = np.ones((128, 1), f32).astype(bf)
    wq, wk, wv, wo = (np.asarray(inputs[k], f32) for k in ["Wq", "Wk", "Wv", "Wo"])
    bq, bv = (np.asarray(inputs[k], f32) for k in ["bq", "bv"])
    in_maps = []
    for b in range(B):
        xtb = np.ascontiguousarray(np.asarray(x_full[b], f32).T).astype(bf)
        for z in range(2):
            sl = slice(z * DO, (z + 1) * DO)
            in_maps.append({
                "xt": xtb,
                "wq": np.ascontiguousarray(wq[:, sl]).astype(bf),
                "wk": np.ascontiguousarray(wk[:, sl]).astype(bf),
                "wv": np.ascontiguousarray(wv[:, sl]).astype(bf),
                "wo": np.ascontiguousarray(wo[sl, :]).astype(bf),
                "bq": np.ascontiguousarray(bq[sl]),
                "bv": np.ascontiguousarray(bv[sl]),
                "mask": mask,
                "ones_c": ones_c,
            })
    return in_maps


def run_cores(cfg, nc, in_maps, bo, trace=False, tmpdir=None):
    c = _derived(cfg)
    n = c["N_CORES"]
    res = run_bass_kernel_spmd(
        nc, in_maps, list(range(n)), trace=trace, tmpdir=tmpdir
    )
    B, T, D = c["B"], c["T"], c["D"]
    out = np.empty((B, T, D), dtype=np.float32)
    bo = np.asarray(bo, np.float32)
    for b in range(B):
        out[b] = res.results[2 * b]["o"] + res.results[2 * b + 1]["o"] + bo
    return out, res


_NC_CACHE = {}


def kernel(x, Wq, bq, Wk, bk, Wv, bv, Wo, bo):
    cfg = PROD_CFG
    key = tuple(sorted(cfg.items()))
    if key not in _NC_CACHE:
        _NC_CACHE[key] = build_nc(cfg)
    nc = _NC_CACHE[key]
    inputs = dict(Wq=Wq, bq=bq, Wk=Wk, bk=bk, Wv=Wv, bv=bv, Wo=Wo, bo=bo)
    in_maps = host_shard(cfg, np.asarray(x, np.float32), inputs)
    out, _ = run_cores(cfg, nc, in_maps, bo)
    return out
